# revision 25
# baseline (speedup 1.0000x reference)
"""CapsNet dynamic-routing kernel for 8 trn2 NeuronCores (pure data parallel).

Math (per batch element b):
  u[n,:]  = squash(W_pc[n] @ x_groups[b,n] + b_pc[n])          n=7 capsules, dim 8
  u_hat[n,m,:] = u[n,:] @ W[n,m]                               m=12 out caps, dim 16
  b_log = 0
  repeat num_iterations:
     c = softmax_m(b_log); s[m] = sum_n c[n,m] u_hat[n,m]; v = squash(s)
     b_log += u_hat . v
  out[m] = |v[m]|

Implementation notes:
  - squash(s) = s|s|/((1+|s|^2)(|s|+eps)); with eps=1e-8 and |s|=O(1) this is
    s/(1+|s|^2) to ~1e-8 relative.
  - softmax without max subtraction (logits bounded, |b| < ~5).
  - iteration 0 has uniform c=1/12 -> s_0 is a static linear map of u, fused
    into the u_hat matmul as 192 extra columns.
  - u_hat columns live in (k, n, m) order (m fastest) so every per-sample
    broadcast multiply has a dense innermost AP and k-reduction is a dense
    contiguous halving tree; n-reduction is a blockwise halving tree.
  - the big elementwise passes run in bf16 (DVE 2x mode; fp32 accumulations
    where it matters) and are split between VectorE and GpSimd with tunable
    cut points (VEC_K / VEC_NM / VEC_CKF); transcendentals, squares and the
    PSUM->SBUF u_hat eviction run on ScalarE.
  - the emission loop is software-pipelined: tile t+1's stage-1 + u_hat
    matmuls are emitted BEFORE tile t's routing so the engine queues stay
    fed across tile boundaries (the Tile scheduler then reorders globally).
  - iteration-0's agreement update is PE-offloaded: with uniform c,
    g0[n,m,j] = sum_k W[n,m,j,k] s0[m,k] is a STATIC linear map of u
    (coefficients (1/12) sum_k W[n,m,j,k] W[n',m,j',k], precomputed on the
    host), emitted as 672 extra columns of the u_hat matmul.  Then
    t0[n,m] = sum_j u[n,j] g0[n,m,j] needs only a 672-wide multiply plus a
    j-halving tree (vs 1344-wide product + 1260-add k-tree).  u itself is
    transposed to batch-major on the PE (identity matmul) for that multiply.
  - compute-engine APs are limited to 3 free dims, hence the [c,k,nm] /
    [(ck),n,m] / per-chunk [k,n,m] view tricks.
  - final |v| = sqrt(nsq)/(1+nsq); sqrts batched in one ACT pass at the end.
  - PSUM budget: z(1) + nsqz(1) + {u_hat+g0 2208-col tile, u-transpose}(5)
    = 7 banks.  Using all 8 banks hard-faulted the exec unit (status 101),
    so the u-transpose PSUM lives inside the psuh pool.
"""

import numpy as np

N_CORES = 8
B_TOTAL = 65536
BP = B_TOTAL // N_CORES          # 8192 samples per core
TILE_F = 512                     # stage-1 free width (batch columns)
N_T512 = BP // TILE_F            # 16
CHUNK = 128                      # routing chunk (batch on partitions)
N_CHUNK = TILE_F // CHUNK        # 4 chunks per 512-tile
N_CAP, D_IN, D_U = 7, 30, 8      # input capsules
M_CAP, D_V = 12, 16              # output capsules
NJ = N_CAP * D_U                 # 56
NMK = N_CAP * M_CAP * D_V        # 1344
MK = M_CAP * D_V                 # 192
NM = N_CAP * M_CAP               # 84
UHW = NMK + MK                   # 1536 = u_hat cols + s1 cols
GW = M_CAP * D_U * N_CAP         # 672 static-g0 cols (m, j, n order)
UHW2 = UHW + GW                  # 2208 total PE columns per chunk

ROUT_BF16 = True                 # routing big-pass dtype (bf16 2x vs f32 1x)

# DVE-vs-GpSimd work split knobs (vector gets the first slice).
VEC_K = 12                       # of D_V=16: k-split for product passes
VEC_NM = 60                      # of NM=84: nm-split for ktree levels
VEC_CKF = 0.75                   # fraction of (NCH*D_V) for ntree levels
VEC_N = 5                        # of N_CAP=7: n-split for c_t / d_t passes
VEC_M = 9                        # of M_CAP=12: m-split for the g0 t-mult
VEC_CM = 36                      # of 48: (c m)-split for jtree levels
VEC_FLAT = 128                   # of CW*NM: flat split for blog add
WAVE = 1                         # routing wave groups per tile

_prog_cache = {}


def _build(num_iterations: int, repeats: int = 1):
    import concourse.bass as bass
    import concourse.bacc as bacc
    import concourse.tile as tile
    from concourse import mybir

    f32 = mybir.dt.float32
    bf16 = mybir.dt.bfloat16
    dt_r = bf16 if ROUT_BF16 else f32
    AX = mybir.AxisListType
    OP = mybir.AluOpType
    ACT = mybir.ActivationFunctionType

    nc = bacc.Bacc()

    xT = nc.declare_dram_parameter("xT", [210, BP], f32, isOutput=False)
    w1 = nc.declare_dram_parameter("w1", [210, NJ], f32, isOutput=False)
    w2e = nc.declare_dram_parameter("w2e", [NJ, UHW2], dt_r, isOutput=False)
    bpc = nc.declare_dram_parameter("bpc", [NJ, 1], f32, isOutput=False)
    bo = nc.declare_dram_parameter("bo", [NJ, NJ], f32, isOutput=False)
    eye = nc.declare_dram_parameter("eye", [NJ, NJ], dt_r, isOutput=False)
    out = nc.declare_dram_parameter("out", [BP, M_CAP], f32, isOutput=True)

    NCH = N_CHUNK

    with tile.TileContext(nc) as tc:
        with (
            nc.allow_low_precision(reason="bf16 big passes; accumulations "
                                          "that matter are kept fp32"),
            tc.tile_pool(name="singles", bufs=1) as singles,
            tc.tile_pool(name="xin", bufs=3) as xin,
            tc.tile_pool(name="s1pool", bufs=2) as s1pool,
            tc.tile_pool(name="uhp", bufs=3) as uhp,
            tc.tile_pool(name="prods", bufs=2) as prods,
            tc.tile_pool(name="trees", bufs=2) as trees,
            tc.tile_pool(name="smalls", bufs=4) as smalls,
            tc.tile_pool(name="psz", bufs=1, space="PSUM") as psz,
            tc.tile_pool(name="psn", bufs=1, space="PSUM") as psn,
            tc.tile_pool(name="g0p", bufs=2) as g0p,
            tc.tile_pool(name="uBp", bufs=2) as uBp,
            tc.tile_pool(name="psuh", bufs=1, space="PSUM") as psuh,
        ):
            # ---- load constants once ----
            w1a_s = singles.tile([128, NJ], f32)
            w1b_s = singles.tile([82, NJ], f32)
            w2e_s = singles.tile([NJ, UHW2], dt_r)
            bpc_s = singles.tile([NJ, 1], f32)
            bo_s = singles.tile([NJ, NJ], f32)
            eye_s = singles.tile([NJ, NJ], dt_r)
            qbuf = singles.tile([CHUNK, BP // CHUNK, M_CAP], f32)
            obuf = singles.tile([CHUNK, BP // CHUNK, M_CAP], f32)
            nc.sync.dma_start(out=w1a_s, in_=w1[0:128, :])
            nc.sync.dma_start(out=w1b_s, in_=w1[128:210, :])
            nc.sync.dma_start(out=w2e_s, in_=w2e[:, :])
            nc.sync.dma_start(out=bpc_s, in_=bpc[:, :])
            nc.sync.dma_start(out=bo_s, in_=bo[:, :])
            nc.sync.dma_start(out=eye_s, in_=eye[:, :])

            def tt2(out_, in0, in1, op, axis, cut):
                # split one op along `axis`: [0,cut) -> VectorE, rest -> GpSimd
                nd = len(out_.shape)

                def sl(a, b):
                    return tuple([slice(None)] * axis + [slice(a, b)] +
                                 [slice(None)] * (nd - axis - 1))
                n = out_.shape[axis]
                nc.vector.tensor_tensor(out=out_[sl(0, cut)],
                                        in0=in0[sl(0, cut)],
                                        in1=in1[sl(0, cut)], op=op)
                if cut < n:
                    nc.gpsimd.tensor_tensor(out=out_[sl(cut, n)],
                                            in0=in0[sl(cut, n)],
                                            in1=in1[sl(cut, n)], op=op)

            def uhat_ck(uhs4):
                # [p, c, k, nm] view of u_hat inside uhs4
                return (uhs4.rearrange("p (c w) -> p c w", c=NCH)
                        [:, :, 0:NMK]
                        .rearrange("p c (k nm) -> p c k nm", k=D_V))

            def uhat_chunk(uhs4, cc):
                # [p, k, n, m] view of one chunk's u_hat
                return (uhs4[:, cc * UHW:cc * UHW + NMK]
                        .rearrange("p (k n m) -> p k n m", k=D_V, n=N_CAP))

            def s1_view(uhs4):
                return (uhs4.rearrange("p (c w) -> p c w", c=NCH)
                        [:, :, NMK:UHW])

            # wave decomposition: routing runs on WAVE independent c-groups
            # so the scheduler can overlap one group's trees with the other's
            # products across the serial iteration chain.
            CW = NCH // WAVE if (NCH % WAVE == 0 and WAVE <= NCH) else NCH

            def ktree(src, w):
                # sum over k: contiguous halving; views [p, cw, k', nm]
                def kv(ap, kk):
                    return ap.rearrange("p (c k nm) -> p c k nm",
                                        c=CW, k=kk)

                def lvl(dst, a, b):
                    tt2(dst, a, b, OP.add, 3, VEC_NM)
                w1_ = trees.tile([CHUNK, CW * 8 * NM], dt_r, tag=f"kt1_{w}")
                lvl(kv(w1_, 8), kv(src, 16)[:, :, 0:8, :],
                    kv(src, 16)[:, :, 8:16, :])
                w2_ = trees.tile([CHUNK, CW * 4 * NM], dt_r, tag=f"kt2_{w}")
                lvl(kv(w2_, 4), kv(w1_, 8)[:, :, 0:4, :],
                    kv(w1_, 8)[:, :, 4:8, :])
                w3_ = trees.tile([CHUNK, CW * 2 * NM], dt_r, tag=f"kt3_{w}")
                lvl(kv(w3_, 2), kv(w2_, 4)[:, :, 0:2, :],
                    kv(w2_, 4)[:, :, 2:4, :])
                tt = smalls.tile([CHUNK, CW * NM], f32, tag=f"t_t{w}")
                nc.vector.tensor_add(kv(tt, 1), kv(w3_, 2)[:, :, 0:1, :],
                                     kv(w3_, 2)[:, :, 1:2, :])
                return tt

            def ntree(pc, w, out_f32=False):
                # sum over n: blockwise halving; views [p, (ck), n', m]
                ckv = int(CW * D_V * VEC_CKF)

                def v(ap, nn):
                    return ap.rearrange("p (ck n m) -> p ck n m",
                                        ck=CW * D_V, n=nn)

                def lvl(dst, a, b):
                    tt2(dst, a, b, OP.add, 1, ckv)
                pcv = v(pc, N_CAP)
                w1_ = trees.tile([CHUNK, CW * D_V * 3 * M_CAP], dt_r,
                                 tag=f"nt1_{w}")
                lvl(v(w1_, 3), pcv[:, :, 0:3, :], pcv[:, :, 3:6, :])
                y = trees.tile([CHUNK, CW * D_V * M_CAP], dt_r,
                               tag=f"nt2_{w}")
                lvl(v(y, 1), v(w1_, 3)[:, :, 0:1, :],
                    v(w1_, 3)[:, :, 1:2, :])
                y2 = trees.tile([CHUNK, CW * D_V * M_CAP], dt_r,
                                tag=f"nt3_{w}")
                lvl(v(y2, 1), v(y, 1), v(w1_, 3)[:, :, 2:3, :])
                st = smalls.tile([CHUNK, CW * D_V * M_CAP],
                                 f32 if out_f32 else dt_r, tag=f"s_t{w}")
                tt2(v(st, 1), v(y2, 1), pcv[:, :, 6:7, :], OP.add, 1, ckv)
                # [p, cw, (k m)]
                return st.rearrange("p (c km) -> p c km", c=CW)

            def nsq_of(s_ap, w):
                # |s|^2 per (chunk, m) from s [p, cw, (k m)]
                sqs = smalls.tile([CHUNK, CW, MK], dt_r, tag=f"sqs{w}")
                nc.scalar.activation(out=sqs, in_=s_ap, func=ACT.Square)
                nsq = smalls.tile([CHUNK, CW * M_CAP], f32, tag=f"nsq{w}")
                nc.vector.tensor_reduce(
                    nsq, sqs.rearrange("p c (k m) -> p c m k", k=D_V),
                    axis=AX.X, op=OP.add)
                p1 = smalls.tile([CHUNK, CW * M_CAP], f32, tag=f"p1{w}")
                nc.scalar.add(p1, nsq, 1.0)
                sh = smalls.tile([CHUNK, CW * M_CAP], f32, tag=f"sh{w}")
                nc.vector.reciprocal(sh, p1)
                return nsq, p1, sh

            def front(t):
                """stage 1 + u_hat matmuls + s1 norms for tile t."""
                c0 = t * TILE_F
                xa = xin.tile([128, TILE_F], f32, tag="xa")
                xb = xin.tile([82, TILE_F], f32, tag="xb")
                nc.sync.dma_start(out=xa, in_=xT[0:128, c0:c0 + TILE_F])
                nc.sync.dma_start(out=xb, in_=xT[128:210, c0:c0 + TILE_F])

                z = psz.tile([NJ, TILE_F], f32)
                nc.tensor.matmul(z, w1a_s, xa, start=True, stop=False)
                nc.tensor.matmul(z, w1b_s, xb, start=False, stop=True)

                # sq = (z + b_pc)^2   (ACT, bias per partition)
                sq = s1pool.tile([NJ, TILE_F], f32, tag="sq")
                nc.scalar.activation(out=sq, in_=z, func=ACT.Square,
                                     bias=bpc_s, scale=1.0)
                # per-capsule |u_raw|^2, replicated across its 8 rows
                nsqz = psn.tile([NJ, TILE_F], f32)
                nc.tensor.matmul(nsqz, bo_s, sq, start=True, stop=True)
                # f = 1/(1+nsq)
                pf = s1pool.tile([NJ, TILE_F], f32, tag="pf")
                nc.scalar.add(pf, nsqz, 1.0)
                fz = s1pool.tile([NJ, TILE_F], f32, tag="fz")
                nc.vector.reciprocal(fz, pf)
                # uT = (z + b_pc) * f
                uTb = s1pool.tile([NJ, TILE_F], dt_r, tag="uT")
                nc.vector.scalar_tensor_tensor(
                    out=uTb, in0=z, scalar=bpc_s, in1=fz,
                    op0=OP.add, op1=OP.mult)

                uhs4 = uhp.tile([CHUNK, NCH * UHW], dt_r, tag="uhs")
                uhs4v = uhs4.rearrange("p (c w) -> p c w", c=NCH)
                g0t = g0p.tile([CHUNK, NCH * GW], dt_r, tag="g0")
                g0v = g0t.rearrange("p (c w) -> p c w", c=NCH)
                uB = uBp.tile([CHUNK, NCH, NJ], dt_r, tag="uB")
                for cc in range(NCH):
                    uh = psuh.tile([CHUNK, UHW2], f32)
                    lhsT = uTb[:, cc * CHUNK:(cc + 1) * CHUNK]
                    for j in range(3):
                        nc.tensor.matmul(uh[:, j * 512:(j + 1) * 512], lhsT,
                                         w2e_s[:, j * 512:(j + 1) * 512],
                                         start=True, stop=True)
                    nc.tensor.matmul(uh[:, UHW:UHW + 512], lhsT,
                                     w2e_s[:, UHW:UHW + 512],
                                     start=True, stop=True)
                    nc.tensor.matmul(uh[:, UHW + 512:UHW2], lhsT,
                                     w2e_s[:, UHW + 512:UHW2],
                                     start=True, stop=True)
                    nc.scalar.copy(uhs4v[:, cc, :], uh[:, 0:UHW])
                    nc.scalar.copy(g0v[:, cc, :], uh[:, UHW:UHW2])
                    # batch-major u for the it0 t-multiply, (j-major, n)
                    ub_ps = psuh.tile([CHUNK, NJ], dt_r, tag="ub")
                    nc.tensor.transpose(ub_ps, lhsT, eye_s)
                    nc.scalar.copy(
                        out=uB[:, cc, :].rearrange("p (j n) -> p n j",
                                                   j=D_U),
                        in_=ub_ps.rearrange("p (n j) -> p n j", j=D_U))

                return (uhs4, g0t, uB)

            def route_w(t, uhs4, g0t, uB, w):
                """routing iterations for wave w (chunks [w*CW,(w+1)*CW))."""
                clo = w * CW
                uck = uhat_ck(uhs4)[:, clo:clo + CW]      # [p, cw, k, nm]
                # iteration-0 norms (uniform c; s1 precomputed by PE)
                s_t = s1_view(uhs4)[:, clo:clo + CW]
                nsq, p1, sh = nsq_of(s_t, w)
                blog = None
                for it in range(num_iterations):
                    last = (it == num_iterations - 1)
                    if it > 0:
                        # c = softmax_m(blog), [p, (c n m)]
                        e = smalls.tile([CHUNK, CW * NM], dt_r, tag=f"e{w}")
                        nc.scalar.activation(out=e, in_=blog, func=ACT.Exp)
                        zs = smalls.tile([CHUNK, CW * N_CAP], f32,
                                         tag=f"zs{w}")
                        nc.vector.tensor_reduce(
                            zs, e.rearrange("p (c n m) -> p c n m",
                                            c=CW, n=N_CAP),
                            axis=AX.X, op=OP.add)
                        rz = smalls.tile([CHUNK, CW * N_CAP], dt_r,
                                         tag=f"rz{w}")
                        nc.vector.reciprocal(rz, zs)
                        c_t = smalls.tile([CHUNK, CW * NM], dt_r,
                                          tag=f"c_t{w}")
                        tt2(c_t.rearrange("p (c n m) -> p c n m",
                                          c=CW, n=N_CAP),
                            e.rearrange("p (c n m) -> p c n m",
                                        c=CW, n=N_CAP),
                            rz.rearrange("p (c n) -> p c n", c=CW)
                              .unsqueeze(3)
                              .broadcast_to([CHUNK, CW, N_CAP, M_CAP]),
                            OP.mult, 2, VEC_N)
                        # s = sum_n c * u_hat   (bcast over k: [c, k, nm])
                        pc = prods.tile([CHUNK, CW * NMK], dt_r,
                                        tag=f"prod{w}")
                        pcv = pc.rearrange("p (c k nm) -> p c k nm",
                                           c=CW, k=D_V)
                        cbc = (c_t.rearrange("p (c nm) -> p c nm", c=CW)
                               .unsqueeze(2)
                               .broadcast_to([CHUNK, CW, D_V, NM]))
                        tt2(pcv, uck, cbc, OP.mult, 2, VEC_K)
                        s_t = ntree(pc, w, out_f32=last)
                        nsq, p1, sh = nsq_of(s_t, w)

                    if not last:
                        if it == 0:
                            # t0 = sum_j u * g0  (g0 = static PE columns)
                            pt0 = trees.tile([CHUNK, CW * GW], dt_r,
                                             tag=f"pt0_{w}")
                            for ci in range(CW):
                                ubc = (uB[:, clo + ci, :]
                                       .rearrange("p (j n) -> p j n", j=D_U)
                                       .unsqueeze(1)
                                       .broadcast_to([CHUNK, M_CAP, D_U,
                                                      N_CAP]))
                                tt2(pt0[:, ci * GW:(ci + 1) * GW]
                                    .rearrange("p (m j n) -> p m j n",
                                               m=M_CAP, j=D_U),
                                    g0t[:, (clo + ci) * GW:
                                        (clo + ci + 1) * GW]
                                    .rearrange("p (m j n) -> p m j n",
                                               m=M_CAP, j=D_U),
                                    ubc, OP.mult, 1, VEC_M)
                            # sum over j: halving tree on [p, (c m), j', n]
                            def jv(ap, jj):
                                return ap.rearrange(
                                    "p (cm j n) -> p cm j n",
                                    cm=CW * M_CAP, j=jj)
                            jt1 = trees.tile([CHUNK, CW * M_CAP * 4 * N_CAP],
                                             dt_r, tag=f"jt1_{w}")
                            tt2(jv(jt1, 4), jv(pt0, 8)[:, :, 0:4, :],
                                jv(pt0, 8)[:, :, 4:8, :], OP.add, 1, VEC_CM)
                            jt2 = trees.tile([CHUNK, CW * M_CAP * 2 * N_CAP],
                                             dt_r, tag=f"jt2_{w}")
                            tt2(jv(jt2, 2), jv(jt1, 4)[:, :, 0:2, :],
                                jv(jt1, 4)[:, :, 2:4, :], OP.add, 1, VEC_CM)
                            t_t = smalls.tile([CHUNK, CW * M_CAP * N_CAP],
                                              f32, tag=f"t0_{w}")
                            tt2(jv(t_t, 1), jv(jt2, 2)[:, :, 0:1, :],
                                jv(jt2, 2)[:, :, 1:2, :], OP.add, 1, VEC_CM)
                            d_t = smalls.tile([CHUNK, CW * NM], f32,
                                              tag=f"d_t{w}")
                            tt2(d_t.rearrange("p (c n m) -> p c n m",
                                              c=CW, n=N_CAP),
                                t_t.rearrange("p (c m n) -> p c n m",
                                              c=CW, m=M_CAP),
                                sh.rearrange("p (c m) -> p c m", c=CW)
                                  .unsqueeze(2)
                                  .broadcast_to([CHUNK, CW, N_CAP, M_CAP]),
                                OP.mult, 2, VEC_N)
                        else:
                            # t = sum_k u_hat*s ; per-chunk [k, n, m] TTs
                            pt = prods.tile([CHUNK, CW * NMK], dt_r,
                                            tag=f"prod{w}")
                            for ci in range(CW):
                                sbc = (s_t[:, ci, :]
                                       .rearrange("p (k m) -> p k m", k=D_V)
                                       .unsqueeze(2)
                                       .broadcast_to([CHUNK, D_V, N_CAP,
                                                      M_CAP]))
                                tt2(pt[:, ci * NMK:(ci + 1) * NMK]
                                    .rearrange("p (k n m) -> p k n m",
                                               k=D_V, n=N_CAP),
                                    uhat_chunk(uhs4, clo + ci), sbc,
                                    OP.mult, 1, VEC_K)
                            t_t = ktree(pt, w)
                            d_t = smalls.tile([CHUNK, CW * NM], f32,
                                              tag=f"d_t{w}")
                            tt2(d_t.rearrange("p (c n m) -> p c n m",
                                              c=CW, n=N_CAP),
                                t_t.rearrange("p (c n m) -> p c n m",
                                              c=CW, n=N_CAP),
                                sh.rearrange("p (c m) -> p c m", c=CW)
                                  .unsqueeze(2)
                                  .broadcast_to([CHUNK, CW, N_CAP, M_CAP]),
                                OP.mult, 2, VEC_N)
                        if it == 0:
                            blog = d_t
                        else:
                            nblog = smalls.tile([CHUNK, CW * NM], f32,
                                                tag=f"blog{w}")
                            tt2(nblog, blog, d_t, OP.add, 1,
                                min(VEC_FLAT, CW * NM))
                            blog = nblog
                    else:
                        # q = nsq*sh^2 ; |v| = sqrt(q) (batched at end)
                        a_t = smalls.tile([CHUNK, CW * M_CAP], f32,
                                          tag=f"a_t{w}")
                        nc.vector.tensor_mul(a_t, nsq, sh)
                        nc.vector.tensor_mul(
                            qbuf[:, t * NCH + clo:t * NCH + clo + CW, :]
                            .rearrange("p c m -> p (c m)"),
                            a_t, sh)

            def route(t, uhs4, g0t, uB):
                for w in range(WAVE):
                    route_w(t, uhs4, g0t, uB, w)

            # ---- software-pipelined emission: front(t+1) before route(t) ----
            seq = [tt % N_T512 for tt in range(N_T512 * repeats)]
            state = front(seq[0])
            for i, t in enumerate(seq):
                nxt = front(seq[i + 1]) if i + 1 < len(seq) else None
                route(t, *state)
                state = nxt

            # ---- batched final sqrt + single output DMA ----
            nc.scalar.activation(out=obuf, in_=qbuf, func=ACT.Sqrt)
            nc.sync.dma_start(
                out=out.rearrange("(g p) m -> p g m", p=CHUNK, g=BP // CHUNK),
                in_=obuf)
    nc.compile()
    return nc


def _prep_weights(W_pc, b_pc, W):
    W1 = np.zeros((210, NJ), np.float32)
    W2E = np.zeros((NJ, UHW2), np.float32)
    BO = np.zeros((NJ, NJ), np.float32)
    for n in range(N_CAP):
        W1[n * D_IN:(n + 1) * D_IN, n * D_U:(n + 1) * D_U] = W_pc[n].T
        BO[n * D_U:(n + 1) * D_U, n * D_U:(n + 1) * D_U] = 1.0
    for n in range(N_CAP):
        for m in range(M_CAP):
            for k in range(D_V):
                # u_hat columns in (k, n, m) order; s1 columns in (k, m)
                W2E[n * D_U:(n + 1) * D_U, k * NM + n * M_CAP + m] = W[n, m, :, k]
                W2E[n * D_U:(n + 1) * D_U, NMK + k * M_CAP + m] += (
                    W[n, m, :, k] / float(M_CAP))
    # static-g0 columns: g0[n,m,j] = sum_k W[n,m,j,k] * s0[m,k] with
    # s0 = mean_n u_hat -> coefficient of u[n',j'] is
    # (1/12) sum_k W[n,m,j,k] W[n',m,j',k].  Column order (m, j, n).
    Q0 = np.einsum('nmjk,NmJk->mjnNJ', W, W).astype(np.float32) / float(M_CAP)
    W2E[:, UHW:UHW2] = Q0.reshape(GW, NJ).T.reshape(NJ, GW)
    BPC = b_pc.reshape(NJ, 1).astype(np.float32)
    return W1, W2E, BO, BPC


def _make_in_maps(x, W_pc, b_pc, W):
    W1, W2E, BO, BPC = _prep_weights(W_pc, b_pc, W)
    EYE = np.eye(NJ, dtype=np.float32)
    if ROUT_BF16:
        import ml_dtypes
        W2E = W2E.astype(ml_dtypes.bfloat16)
        EYE = EYE.astype(ml_dtypes.bfloat16)
    xt = np.ascontiguousarray(x.T)                      # [210, B]
    in_maps = []
    for i in range(N_CORES):
        in_maps.append({
            "xT": np.ascontiguousarray(xt[:, i * BP:(i + 1) * BP]),
            "w1": W1, "w2e": W2E, "bpc": BPC, "bo": BO, "eye": EYE,
        })
    return in_maps


def kernel(x, W_pc, b_pc, W, num_iterations, _trace=False):
    from concourse.bass_utils import run_bass_kernel_spmd

    x = np.asarray(x, np.float32)
    W_pc = np.asarray(W_pc, np.float32)
    b_pc = np.asarray(b_pc, np.float32)
    W = np.asarray(W, np.float32)
    nit = int(num_iterations)
    assert x.shape == (B_TOTAL, 210)

    key = nit
    if key not in _prog_cache:
        _prog_cache[key] = _build(nit)
    nc = _prog_cache[key]

    in_maps = _make_in_maps(x, W_pc, b_pc, W)
    res = run_bass_kernel_spmd(nc, in_maps, list(range(N_CORES)),
                               trace=_trace)
    outs = [res.results[i]["out"] for i in range(N_CORES)]
    full = np.concatenate(outs, axis=0)
    if _trace:
        kernel._last_exec_time_ns = res.exec_time_ns
        kernel._last_results = res
    return full
